# revision 1
# baseline (speedup 1.0000x reference)
"""Trainium2 Bass kernel for a 4-layer LIF spiking net (BPSpikingNet).

Reference semantics (per timestep t, per layer l):
    i = h @ W_l.T + b_l
    v = v - v/tau + i          (tau=2  ->  v = 0.5*v + i)
    s = (v >= 1.0)
    v = (1-s) * v              (hard reset to 0)
    h = s
Output = layer-4 spike train, shape [T=32, B=128, 1000], fp32.

Strategy:
  * Data-parallel over batch: B=128 -> 16 samples per core across 8 cores.
  * Layer-by-layer: layer l's matmul input (spikes of l-1) is fully known
    once l-1's recurrence is done, so each layer is ONE dense GEMM over all
    T*Bs = 512 (t,b) columns (neuron-major / weight-stationary, N=512 moving),
    followed by a 32-step elementwise LIF recurrence on [128, O*16] tiles.
  * bf16 matmuls (spikes are exact in bf16; weight rounding is far below the
    spiking threshold margin), fp32 PSUM accumulate, fp32 recurrence.
  * Recurrence: charge writes the charged potential in-place into the current
    buffer iT[:, t] (2 DVE ops per step on the serial chain), and spikes for
    ALL timesteps are extracted afterwards with a single big is_ge op.
"""

import numpy as np
import ml_dtypes

T = 32
B = 128
NCORES = 8
BS = B // NCORES          # 16 samples per core
COLS = T * BS             # 512 (t,b) columns per core
NIN = 2048
KT = NIN // 128           # 16 k-tiles (all layers have 2048 inputs)
O_LIST = [16, 16, 16, 8]  # output 128-tiles per layer (layer 4 padded 1000->1024)
BOFF = [0, 16, 32, 48]    # bias column offset per layer
NB = sum(O_LIST)          # 56 bias columns

_CACHE = {}

TRACE = False             # set True (from test.py) to capture an NTFF profile
LAST_RESULTS = None       # BassKernelResults of the most recent run
EVICT_ENGINE = "scalar"   # "scalar" (ACT Identity+bias) or "vector" fallback


def _build_nc():
    import concourse.mybir as mybir
    import concourse.tile as tile
    from concourse import bacc

    dt = mybir.dt
    alu = mybir.AluOpType

    nc = bacc.Bacc("TRN2", target_bir_lowering=False, debug=False,
                   num_devices=NCORES)

    x_d = nc.dram_tensor("x", [128, KT, COLS], dt.bfloat16, kind="ExternalInput")
    w_d = [
        nc.dram_tensor(f"w{li}", [O_LIST[li], 128, KT, 128], dt.bfloat16,
                       kind="ExternalInput")
        for li in range(4)
    ]
    b_d = nc.dram_tensor("bias", [128, NB], dt.float32, kind="ExternalInput")
    out_d = nc.dram_tensor("out", [128, T, O_LIST[3], BS], dt.bfloat16,
                           kind="ExternalOutput")

    TH = T // 2           # 16 timesteps per half
    HC = TH * BS          # 256 columns per half

    with tile.TileContext(nc) as tc:
        with (
            tc.tile_pool(name="xp", bufs=1) as xp,
            tc.tile_pool(name="sp", bufs=1) as sp,
            tc.tile_pool(name="ip", bufs=2) as ip,
            tc.tile_pool(name="wp", bufs=6) as wp,
            tc.tile_pool(name="vp", bufs=1) as vp,
            tc.tile_pool(name="bp", bufs=1) as bp,
            tc.tile_pool(name="ps", bufs=4, space="PSUM") as ps,
        ):
            # x in 8 chunks on the gpsimd DMA queue (weights go on sync's),
            # so the first matmul's two dependencies transfer in parallel
            xq = []
            for c in range(8):
                xc = xp.tile([128, 2, COLS], dt.bfloat16, tag=f"x{c}")
                nc.gpsimd.dma_start(xc[:], x_d.ap()[:, 2 * c:2 * c + 2, :])
                xq.append(xc)
            bt = bp.tile([128, NB], dt.float32)
            nc.gpsimd.dma_start(bt[:], b_d.ap())

            # PE warmup: ~60 junk matmuls on a zeroed scratch tile while the
            # first DMAs land, so the HAM clock gate opens (1.2->2.4 GHz)
            # before real work arrives. Results go to a scratch PSUM bank
            # that is never read.
            wu = xp.tile([128, 128], dt.bfloat16, tag="warm")
            nc.vector.memset(wu[:], 0.0)
            wacc = ps.tile([128, 128], dt.float32, tag="wacc")
            for _ in range(60):
                nc.tensor.matmul(wacc[:], wu[:], wu[:], start=True, stop=True)

            its = [None] * 4
            sts = [None] * 4
            vbs = [None] * 4

            def gemm_half(li, h):
                O = O_LIST[li]
                it = its[li]
                for o in range(O):
                    wt = wp.tile([128, KT, 128], dt.bfloat16, tag="wt")
                    if li == 0 and h == 0 and o == 0:
                        # split the very first weight DMA so matmul 0 starts
                        # after half the tile has landed
                        nc.sync.dma_start(wt[:, :KT // 2], w_d[0].ap()[0, :, :KT // 2])
                        nc.sync.dma_start(wt[:, KT // 2:], w_d[0].ap()[0, :, KT // 2:])
                    else:
                        nc.sync.dma_start(wt[:], w_d[li].ap()[o])
                    acc = ps.tile([128, HC], dt.float32, tag="acc")
                    for k in range(KT):
                        if li == 0:
                            rhs = xq[k // 2][:, k % 2, h * HC:(h + 1) * HC]
                        else:
                            rhs = sts[li - 1][:, h * TH:(h + 1) * TH, k, :]
                        nc.tensor.matmul(acc[:], wt[:, k, :], rhs,
                                         start=(k == 0), stop=(k == KT - 1))
                    # PSUM -> SBUF eviction with bias add, scattered to t-major
                    bias_ap = bt[:, BOFF[li] + o:BOFF[li] + o + 1]
                    src = acc.rearrange("p (t b) -> p t b", t=TH)
                    dst = it[:, h * TH:(h + 1) * TH, o, :]
                    if EVICT_ENGINE == "scalar":
                        nc.scalar.activation(
                            dst, src, mybir.ActivationFunctionType.Identity,
                            bias=bias_ap, scale=1.0)
                    else:
                        nc.vector.tensor_scalar(dst, src, bias_ap, None, alu.add)

            def rec_half(li, h):
                # charge in place (iT[:,t] becomes the charged potential v(t));
                # only the reset state vb carries between steps
                it, vb = its[li], vbs[li]
                for t in range(h * TH, (h + 1) * TH):
                    nc.vector.scalar_tensor_tensor(
                        it[:, t], vb[:], 0.5, it[:, t], alu.mult, alu.add)
                    nc.vector.scalar_tensor_tensor(
                        vb[:], it[:, t], 1.0, it[:, t], alu.is_lt, alu.mult)
                    if li == 3 and t == h * TH + TH // 2 - 1:
                        # output layer: extract+ship the finished quarter while
                        # the chain continues, so the tail only waits on 8 steps
                        ql = slice(h * TH, t + 1)
                        nc.vector.tensor_scalar(
                            sts[3][:, ql], it[:, ql], 1.0, None, alu.is_ge)
                        nc.sync.dma_start(out_d.ap()[:, ql], sts[3][:, ql])
                if li == 3:
                    ql = slice(h * TH + TH // 2, (h + 1) * TH)
                    nc.vector.tensor_scalar(
                        sts[3][:, ql], it[:, ql], 1.0, None, alu.is_ge)
                    nc.sync.dma_start(out_d.ap()[:, ql], sts[3][:, ql])
                else:
                    sl = slice(h * TH, (h + 1) * TH)
                    nc.vector.tensor_scalar(
                        sts[li][:, sl], it[:, sl], 1.0, None, alu.is_ge)

            for li in range(4):
                O = O_LIST[li]
                its[li] = ip.tile([128, T, O, BS], dt.float32, tag="it",
                                  name=f"it{li}")
                sts[li] = sp.tile([128, T, O, BS], dt.bfloat16, tag=f"s{li}",
                                  name=f"s{li}")
                vbs[li] = vp.tile([128, O, BS], dt.float32, tag=f"vb{li}",
                                  name=f"vb{li}")
                nc.vector.memset(vbs[li][:], 0.0)
                # pipeline: gemm(li,h1); gemm(li,h2) || rec(li,h1);
                # next layer's gemm h1 || rec(li,h2)
                gemm_half(li, 0)
                gemm_half(li, 1)
                rec_half(li, 0)
                rec_half(li, 1)

    nc.compile()
    return nc


def _get_nc():
    if "nc" not in _CACHE:
        _CACHE["nc"] = _build_nc()
    return _CACHE["nc"]


def _host_inputs(x_tbf, Ws, bs):
    """Shared (weight/bias) arrays + per-core x shards, pre-laid-out."""
    bf16 = ml_dtypes.bfloat16
    w_arrs = []
    b_cols = []
    for li in range(4):
        W = np.asarray(Ws[li], np.float32)
        b = np.asarray(bs[li], np.float32)
        O = O_LIST[li]
        if W.shape[0] < O * 128:           # pad layer 4: 1000 -> 1024
            pad = O * 128 - W.shape[0]
            W = np.concatenate([W, np.zeros((pad, NIN), np.float32)], 0)
            b = np.concatenate([b, np.zeros(pad, np.float32)])
        # warr[o, ki, k, mo] = W[o*128+mo, k*128+ki]
        w_arrs.append(np.ascontiguousarray(
            W.reshape(O, 128, KT, 128).transpose(0, 3, 2, 1)).astype(bf16))
        b_cols.append(b.reshape(O, 128))
    b_all = np.ascontiguousarray(np.concatenate(b_cols, 0).T).astype(np.float32)

    x = np.asarray(x_tbf, np.float32)
    x_shards = []
    for c in range(NCORES):
        xc = x[:, c * BS:(c + 1) * BS, :]                    # [T, BS, NIN]
        xc = xc.transpose(2, 0, 1).reshape(NIN, COLS)        # [n, t*BS+b]
        xc = xc.reshape(KT, 128, COLS).transpose(1, 0, 2)    # [p, k, cols]
        x_shards.append(np.ascontiguousarray(xc).astype(bf16))
    return w_arrs, b_all, x_shards


def _decode_out(oc):
    """[128, T, 8, BS] (p,t,o,b) -> [T, BS, 1000] fp32."""
    oc = np.asarray(oc).astype(np.float32)
    oc = oc.transpose(1, 3, 2, 0).reshape(T, BS, O_LIST[3] * 128)
    return oc[:, :, :1000]


def kernel(x_tbf, W1, b1, W2, b2, W3, b3, W4, b4):
    global LAST_RESULTS
    from concourse.bass_utils import run_bass_kernel_spmd

    nc = _get_nc()
    w_arrs, b_all, x_shards = _host_inputs(
        x_tbf, [W1, W2, W3, W4], [b1, b2, b3, b4])

    in_maps = []
    for c in range(NCORES):
        m = {"x": x_shards[c], "bias": b_all}
        for li in range(4):
            m[f"w{li}"] = w_arrs[li]
        in_maps.append(m)

    res = run_bass_kernel_spmd(nc, in_maps, core_ids=list(range(NCORES)),
                               trace=TRACE)
    LAST_RESULTS = res

    out = np.empty((T, B, 1000), np.float32)
    for c in range(NCORES):
        out[:, c * BS:(c + 1) * BS, :] = _decode_out(res.results[c]["out"])
    return out



# revision 5
# speedup vs baseline: 1.5533x; 1.5533x over previous
"""Trainium2 Bass kernel for a 4-layer LIF spiking net (BPSpikingNet).

Reference semantics (per timestep t, per layer l):
    i = h @ W_l.T + b_l
    v = v - v/tau + i          (tau=2  ->  v = 0.5*v + i)
    s = (v >= 1.0)
    v = (1-s) * v              (hard reset to 0)
    h = s
Output = layer-4 spike train, shape [T=32, B=128, 1000], fp32.

Strategy:
  * Data-parallel over batch: B=128 -> 16 samples per core across 8 cores.
  * Layer-by-layer: layer l's matmul input (spikes of l-1) is fully known
    once l-1's recurrence is done, so each layer is ONE dense GEMM over all
    T*Bs = 512 (t,b) columns (weight-stationary), followed by a 32-step
    elementwise LIF recurrence.
  * fp8e4 (e4m3) matmuls in DoubleRow perf mode: each instruction contracts
    TWO 128-row k-tiles (lhsT [128,2,128], rhs [128,2,256]) at 2x PE
    throughput. Spikes are exact in fp8; weights are pre-scaled x8 on the
    host (better e4m3 coverage of the xavier range) and the PSUM eviction
    applies the inverse scale 1/8. fp32 PSUM accumulate.
  * k-major state layout [128, O, T, BS]: PSUM eviction is a flat [128,256]
    ACT copy (bias + 0.125 scale), and the next layer's DoubleRow rhs is a
    clean [128, 2, 256] slice of the fp8 spike tile.
  * fp16 recurrence: charge writes the charged potential in-place into
    it[:, :, t] (2 DVE scalar_tensor_tensor ops per step, 4x_2p mode), and
    spikes for a whole half are extracted afterwards with one is_ge op.
"""

import numpy as np
import ml_dtypes

T = 32
B = 128
NCORES = 8
BS = B // NCORES          # 16 samples per core
COLS = T * BS             # 512 (t,b) columns per core
NIN = 2048
KT2 = NIN // 256          # 8 k-PAIRS (DoubleRow: 2 k-tiles per matmul)
O_LIST = [16, 16, 16, 8]  # output 128-tiles per layer (layer 4 padded 1000->1024)
BOFF = [0, 16, 32, 48]    # bias column offset per layer
NB = sum(O_LIST)          # 56 bias columns
WSCALE = 8.0              # host-side weight scale (inverse applied at eviction)

_CACHE = {}

TRACE = False             # set True (from test.py) to capture an NTFF profile
LAST_RESULTS = None       # BassKernelResults of the most recent run
DEBUG_SPIKES = False      # sim-only: dump layer-1/2 spike tiles to dram


def _build_nc():
    import concourse.mybir as mybir
    import concourse.tile as tile
    from concourse import bacc

    dt = mybir.dt
    alu = mybir.AluOpType
    DR = mybir.MatmulPerfMode.DoubleRow

    nc = bacc.Bacc("TRN2", target_bir_lowering=False, debug=False,
                   num_devices=NCORES)

    x_d = nc.dram_tensor("x", [128, KT2, 2, COLS], dt.float8e4,
                         kind="ExternalInput")
    w_d = [
        nc.dram_tensor(f"w{li}", [O_LIST[li], 128, KT2, 2, 128], dt.float8e4,
                       kind="ExternalInput")
        for li in range(4)
    ]
    b_d = nc.dram_tensor("bias", [128, NB], dt.float32, kind="ExternalInput")
    out_d = nc.dram_tensor("out", [128, O_LIST[3], T, BS], dt.float8e4,
                           kind="ExternalOutput")
    dbg_d = None
    if DEBUG_SPIKES:
        dbg_d = [nc.dram_tensor(f"dbg{li}", [128, O_LIST[li], T, BS],
                                dt.float8e4, kind="ExternalOutput")
                 for li in range(2)]

    TH = T // 2           # 16 timesteps per half
    HC = TH * BS          # 256 columns per half

    with tile.TileContext(nc) as tc:
        with (
            tc.tile_pool(name="xp", bufs=1) as xp,
            tc.tile_pool(name="sp", bufs=1) as sp,
            tc.tile_pool(name="ip", bufs=2) as ip,
            tc.tile_pool(name="wp", bufs=6) as wp,
            tc.tile_pool(name="vp", bufs=1) as vp,
            tc.tile_pool(name="bp", bufs=1) as bp,
            tc.tile_pool(name="ps", bufs=4, space="PSUM") as ps,
        ):
            # x in 8 chunks (one per k-pair) on the gpsimd DMA queue
            # (weights go on sync's), so the first matmul's two dependencies
            # transfer in parallel
            xq = []
            for c in range(KT2):
                xc = xp.tile([128, 2, COLS], dt.float8e4, tag=f"x{c}")
                nc.gpsimd.dma_start(xc[:], x_d.ap()[:, c])
                xq.append(xc)
            bt = bp.tile([128, NB], dt.float32)
            nc.gpsimd.dma_start(bt[:], b_d.ap())

            # PE warmup: junk DoubleRow matmuls on a zeroed scratch tile while
            # the first DMAs land, so the HAM clock gate opens before real
            # work arrives. Results go to a scratch PSUM bank never read.
            wu = xp.tile([128, 2, 128], dt.float8e4, tag="warm")
            nc.vector.memset(wu[:], 0.0)
            wacc = ps.tile([128, 128], dt.float32, tag="wacc")
            for _ in range(60):
                nc.tensor.matmul(wacc[:], wu[:], wu[:], start=True, stop=True,
                                 perf_mode=DR)

            its = [None] * 4
            sts = [None] * 4
            vbs = [None] * 4

            def gemm_half(li, h):
                O = O_LIST[li]
                it = its[li]
                for o in range(O):
                    wt = wp.tile([128, KT2, 2, 128], dt.float8e4, tag="wt")
                    if li == 0 and h == 0 and o == 0:
                        # split the very first weight DMA so matmul 0 starts
                        # after half the tile has landed
                        nc.sync.dma_start(wt[:, :KT2 // 2],
                                          w_d[0].ap()[0, :, :KT2 // 2])
                        nc.sync.dma_start(wt[:, KT2 // 2:],
                                          w_d[0].ap()[0, :, KT2 // 2:])
                    else:
                        nc.sync.dma_start(wt[:], w_d[li].ap()[o])
                    acc = ps.tile([128, HC], dt.float32, tag="acc")
                    for kk in range(KT2):
                        if li == 0:
                            rhs = xq[kk][:, :, h * HC:(h + 1) * HC]
                        else:
                            rhs = sts[li - 1][:, 2 * kk:2 * kk + 2,
                                              h * TH:(h + 1) * TH, :]
                        nc.tensor.matmul(acc[:], wt[:, kk], rhs,
                                         start=(kk == 0), stop=(kk == KT2 - 1),
                                         perf_mode=DR)
                    # PSUM -> SBUF eviction: flat [128, 256] copy with bias add
                    # and the 1/WSCALE de-scale, cast to fp16
                    bias_ap = bt[:, BOFF[li] + o:BOFF[li] + o + 1]
                    nc.scalar.activation(
                        it[:, o, h * TH:(h + 1) * TH, :], acc[:],
                        mybir.ActivationFunctionType.Identity,
                        bias=bias_ap, scale=1.0 / WSCALE)

            def rec_half(li, h):
                # charge in place (it[:, :, t] becomes the charged potential);
                # only the reset state vb carries between steps
                it, vb = its[li], vbs[li]
                for t in range(h * TH, (h + 1) * TH):
                    nc.vector.scalar_tensor_tensor(
                        it[:, :, t], vb[:], 0.5, it[:, :, t], alu.mult, alu.add)
                    nc.vector.scalar_tensor_tensor(
                        vb[:], it[:, :, t], 1.0, it[:, :, t], alu.is_lt, alu.mult)
                    if li == 3 and t == h * TH + TH // 2 - 1:
                        # output layer: extract+ship the finished quarter while
                        # the chain continues, so the tail only waits on 8 steps
                        ql = slice(h * TH, t + 1)
                        nc.vector.tensor_scalar(
                            sts[3][:, :, ql], it[:, :, ql], 1.0, None, alu.is_ge)
                        nc.sync.dma_start(out_d.ap()[:, :, ql], sts[3][:, :, ql])
                if li == 3:
                    ql = slice(h * TH + TH // 2, (h + 1) * TH)
                    nc.vector.tensor_scalar(
                        sts[3][:, :, ql], it[:, :, ql], 1.0, None, alu.is_ge)
                    nc.sync.dma_start(out_d.ap()[:, :, ql], sts[3][:, :, ql])
                else:
                    sl = slice(h * TH, (h + 1) * TH)
                    nc.vector.tensor_scalar(
                        sts[li][:, :, sl], it[:, :, sl], 1.0, None, alu.is_ge)

            for li in range(4):
                O = O_LIST[li]
                its[li] = ip.tile([128, O, T, BS], dt.float16, tag="it",
                                  name=f"it{li}")
                sts[li] = sp.tile([128, O, T, BS], dt.float8e4, tag=f"s{li}",
                                  name=f"s{li}")
                vbs[li] = vp.tile([128, O, BS], dt.float16, tag=f"vb{li}",
                                  name=f"vb{li}")
                nc.vector.memset(vbs[li][:], 0.0)
                # pipeline: gemm(li,h1); gemm(li,h2) || rec(li,h1);
                # next layer's gemm h1 || rec(li,h2)
                gemm_half(li, 0)
                gemm_half(li, 1)
                rec_half(li, 0)
                rec_half(li, 1)

            if DEBUG_SPIKES:
                for li in range(2):
                    nc.sync.dma_start(dbg_d[li].ap(), sts[li][:])

    nc.compile()
    return nc


def _get_nc():
    if "nc" not in _CACHE:
        _CACHE["nc"] = _build_nc()
    return _CACHE["nc"]


def _host_inputs(x_tbf, Ws, bs):
    """Shared (weight/bias) arrays + per-core x shards, pre-laid-out."""
    f8 = ml_dtypes.float8_e4m3
    w_arrs = []
    b_cols = []
    for li in range(4):
        W = np.asarray(Ws[li], np.float32)
        b = np.asarray(bs[li], np.float32)
        O = O_LIST[li]
        if W.shape[0] < O * 128:           # pad layer 4: 1000 -> 1024
            pad = O * 128 - W.shape[0]
            W = np.concatenate([W, np.zeros((pad, NIN), np.float32)], 0)
            b = np.concatenate([b, np.zeros(pad, np.float32)])
        # warr[o, p, kk, i, m] = WSCALE * W[o*128+m, (2kk+i)*128+p]
        w_arrs.append(np.ascontiguousarray(
            (WSCALE * W).reshape(O, 128, KT2, 2, 128)
            .transpose(0, 4, 2, 3, 1)).astype(f8))
        b_cols.append(b.reshape(O, 128))
    b_all = np.ascontiguousarray(np.concatenate(b_cols, 0).T).astype(np.float32)

    x = np.asarray(x_tbf, np.float32)
    x_shards = []
    for c in range(NCORES):
        xc = x[:, c * BS:(c + 1) * BS, :]                    # [T, BS, NIN]
        xc = xc.transpose(2, 0, 1).reshape(NIN, COLS)        # [n, t*BS+b]
        # [p, kk, i, cols]
        xc = xc.reshape(KT2, 2, 128, COLS).transpose(2, 0, 1, 3)
        x_shards.append(np.ascontiguousarray(xc).astype(f8))
    return w_arrs, b_all, x_shards


def _decode_out(oc):
    """[128, 8, T, BS] (m, o, t, b) fp8 -> [T, BS, 1000] fp32."""
    oc = np.asarray(oc).astype(np.float32)
    oc = oc.transpose(2, 3, 1, 0).reshape(T, BS, O_LIST[3] * 128)
    return oc[:, :, :1000]


def kernel(x_tbf, W1, b1, W2, b2, W3, b3, W4, b4):
    global LAST_RESULTS
    from concourse.bass_utils import run_bass_kernel_spmd

    nc = _get_nc()
    w_arrs, b_all, x_shards = _host_inputs(
        x_tbf, [W1, W2, W3, W4], [b1, b2, b3, b4])

    in_maps = []
    for c in range(NCORES):
        m = {"x": x_shards[c], "bias": b_all}
        for li in range(4):
            m[f"w{li}"] = w_arrs[li]
        in_maps.append(m)

    res = run_bass_kernel_spmd(nc, in_maps, core_ids=list(range(NCORES)),
                               trace=TRACE)
    LAST_RESULTS = res

    out = np.empty((T, B, 1000), np.float32)
    for c in range(NCORES):
        out[:, c * BS:(c + 1) * BS, :] = _decode_out(res.results[c]["out"])
    return out


# revision 8
# speedup vs baseline: 1.7154x; 1.1044x over previous
"""Trainium2 Bass kernel for a 4-layer LIF spiking net (BPSpikingNet).

Reference semantics (per timestep t, per layer l):
    i = h @ W_l.T + b_l
    v = v - v/tau + i          (tau=2  ->  v = 0.5*v + i)
    s = (v >= 1.0)
    v = (1-s) * v              (hard reset to 0)
    h = s
Output = layer-4 spike train, shape [T=32, B=128, 1000], fp32.

Strategy:
  * Data-parallel over batch: B=128 -> 16 samples per core across 8 cores.
  * Layer-by-layer: layer l's matmul input (spikes of l-1) is fully known
    once l-1's recurrence is done, so each layer is ONE dense GEMM over all
    T*Bs = 512 (t,b) columns (weight-stationary), followed by a 32-step
    elementwise LIF recurrence.
  * fp8e4 (e4m3) matmuls in DoubleRow perf mode: each instruction contracts
    TWO 128-row k-tiles (lhsT [128,2,128], rhs [128,2,256]) at 2x PE
    throughput. Spikes are exact in fp8; weights are pre-scaled x8 on the
    host (better e4m3 coverage of the xavier range) and the PSUM eviction
    applies the inverse scale 1/8. fp32 PSUM accumulate.
  * k-major state layout [128, O, T, BS]: PSUM eviction is a flat [128,256]
    ACT copy (bias + 0.125 scale), and the next layer's DoubleRow rhs is a
    clean [128, 2, 256] slice of the fp8 spike tile.
  * fp16 recurrence: charge writes the charged potential in-place into
    it[:, :, t] (2 DVE scalar_tensor_tensor ops per step, 4x_2p mode), and
    spikes for a whole half are extracted afterwards with one is_ge op.
"""

import numpy as np
import ml_dtypes

T = 32
B = 128
NCORES = 8
BS = B // NCORES          # 16 samples per core
COLS = T * BS             # 512 (t,b) columns per core
NIN = 2048
KT2 = NIN // 256          # 8 k-PAIRS (DoubleRow: 2 k-tiles per matmul)
O_LIST = [16, 16, 16, 8]  # output 128-tiles per layer (layer 4 padded 1000->1024)
BOFF = [0, 16, 32, 48]    # bias column offset per layer
NB = sum(O_LIST)          # 56 bias columns
WSCALE = 8.0              # host-side weight scale (inverse applied at eviction)

_CACHE = {}

TRACE = False             # set True (from test.py) to capture an NTFF profile
LAST_RESULTS = None       # BassKernelResults of the most recent run
DEBUG_SPIKES = False      # sim-only: dump layer-1/2 spike tiles to dram


def _build_nc():
    import concourse.mybir as mybir
    import concourse.tile as tile
    from concourse import bacc

    dt = mybir.dt
    alu = mybir.AluOpType
    DR = mybir.MatmulPerfMode.DoubleRow

    nc = bacc.Bacc("TRN2", target_bir_lowering=False, debug=False,
                   num_devices=NCORES)

    x_d = nc.dram_tensor("x", [128, KT2, 2, COLS], dt.float8e4,
                         kind="ExternalInput")
    w_d = [
        nc.dram_tensor(f"w{li}", [O_LIST[li], 128, KT2, 2, 128], dt.float8e4,
                       kind="ExternalInput")
        for li in range(4)
    ]
    b_d = nc.dram_tensor("bias", [128, NB], dt.float32, kind="ExternalInput")
    out_d = nc.dram_tensor("out", [128, O_LIST[3], T, BS], dt.float8e4,
                           kind="ExternalOutput")
    dbg_d = None
    if DEBUG_SPIKES:
        dbg_d = [nc.dram_tensor(f"dbg{li}", [128, O_LIST[li], T, BS],
                                dt.float8e4, kind="ExternalOutput")
                 for li in range(2)]

    TH = T // 2           # 16 timesteps per half
    HC = TH * BS          # 256 columns per half

    with tile.TileContext(nc) as tc:
        with (
            tc.tile_pool(name="xp", bufs=1) as xp,
            tc.tile_pool(name="sp", bufs=1) as sp,
            tc.tile_pool(name="ip", bufs=2) as ip,
            tc.tile_pool(name="wp", bufs=6) as wp,
            tc.tile_pool(name="vp", bufs=1) as vp,
            tc.tile_pool(name="bp", bufs=1) as bp,
            tc.tile_pool(name="ps", bufs=4, space="PSUM") as ps,
        ):
            # x in 8 chunks (one per k-pair) on the gpsimd DMA queue
            # (weights go on sync's), so the first matmul's two dependencies
            # transfer in parallel
            xq = []
            for c in range(KT2):
                xc = xp.tile([128, 2, COLS], dt.float8e4, tag=f"x{c}")
                nc.gpsimd.dma_start(xc[:], x_d.ap()[:, c])
                xq.append(xc)
            bt = bp.tile([128, NB], dt.float32)
            nc.gpsimd.dma_start(bt[:], b_d.ap())

            # PE warmup: junk DoubleRow matmuls on a zeroed scratch tile while
            # the first DMAs land, so the HAM clock gate opens before real
            # work arrives. Results go to a scratch PSUM bank never read.
            wu = xp.tile([128, 2, 128], dt.float8e4, tag="warm")
            nc.vector.memset(wu[:], 0.0)
            wacc = ps.tile([128, 128], dt.float32, tag="wacc")
            for _ in range(30):
                nc.tensor.matmul(wacc[:], wu[:], wu[:], start=True, stop=True,
                                 perf_mode=DR)
            # DVE warmup: junk chain ops so the first real recurrence runs at
            # the ramped clock (DVE would otherwise idle until ~16us).
            wv = xp.tile([128, 2, 128], dt.float16, tag="warmv")
            nc.vector.memset(wv[:], 0.0)
            for _ in range(28):
                nc.vector.scalar_tensor_tensor(
                    wv[:, 0], wv[:, 1], 0.5, wv[:, 0], alu.mult, alu.add)

            its = [None] * 4
            sts = [None] * 4
            vbs = [None] * 4

            def gemm_half(li, h):
                O = O_LIST[li]
                it = its[li]
                for o in range(O):
                    wt = wp.tile([128, KT2, 2, 128], dt.float8e4, tag="wt")
                    if li == 0 and h == 0 and o == 0:
                        # split the very first weight DMA so matmul 0 starts
                        # after half the tile has landed
                        nc.sync.dma_start(wt[:, :KT2 // 2],
                                          w_d[0].ap()[0, :, :KT2 // 2])
                        nc.sync.dma_start(wt[:, KT2 // 2:],
                                          w_d[0].ap()[0, :, KT2 // 2:])
                    else:
                        nc.sync.dma_start(wt[:], w_d[li].ap()[o])
                    acc = ps.tile([128, HC], dt.float32, tag="acc")
                    for kk in range(KT2):
                        if li == 0:
                            rhs = xq[kk][:, :, h * HC:(h + 1) * HC]
                        else:
                            rhs = sts[li - 1][:, 2 * kk:2 * kk + 2,
                                              h * TH:(h + 1) * TH, :]
                        nc.tensor.matmul(acc[:], wt[:, kk], rhs,
                                         start=(kk == 0), stop=(kk == KT2 - 1),
                                         perf_mode=DR)
                    # PSUM -> SBUF eviction: flat [128, 256] copy with bias add
                    # and the 1/WSCALE de-scale, cast to fp16
                    bias_ap = bt[:, BOFF[li] + o:BOFF[li] + o + 1]
                    nc.scalar.activation(
                        it[:, o, h * TH:(h + 1) * TH, :], acc[:],
                        mybir.ActivationFunctionType.Identity,
                        bias=bias_ap, scale=1.0 / WSCALE)

            def rec_half(li, h):
                # charge in place (it[:, :, t] becomes the charged potential);
                # only the reset state vb carries between steps
                it, vb = its[li], vbs[li]
                # Guard: a 1-column no-op (it_c = 0*s_c + it_c) that makes this
                # chain's first touch of it[:, :, first_t] READ the previous
                # extraction's output. The tile scheduler otherwise interleaves
                # this chain's ops (which wait on this gemm's evictions) into
                # the previous chain on the in-order DVE queue, head-of-line
                # blocking the previous extraction and stalling the PE.
                if li > 0 or h > 0:
                    if h == 1:
                        # previous DVE milestone: this layer's h0 extraction
                        g_src = sts[li][:, 0, TH // 2 if li == 3 else 0, 0:1]
                    else:
                        # previous layer's h1 extraction
                        g_src = sts[li - 1][:, 0, TH, 0:1]
                    g_dst = it[:, 0, h * TH, 0:1]
                    nc.vector.scalar_tensor_tensor(
                        g_dst, g_src, 0.0, g_dst, alu.mult, alu.add)
                for t in range(h * TH, (h + 1) * TH):
                    if t > 0:
                        nc.vector.scalar_tensor_tensor(
                            it[:, :, t], vb[:], 0.5, it[:, :, t], alu.mult, alu.add)
                    # t == 0: v=0, so the charged potential is i_0 (already in
                    # place); the reset below also initializes vb (no memset).
                    nc.vector.scalar_tensor_tensor(
                        vb[:], it[:, :, t], 1.0, it[:, :, t], alu.is_lt, alu.mult)
                    if li == 3 and t == h * TH + TH // 2 - 1:
                        # output layer: extract+ship the finished quarter while
                        # the chain continues, so the tail only waits on 8 steps
                        ql = slice(h * TH, t + 1)
                        nc.vector.tensor_scalar(
                            sts[3][:, :, ql], it[:, :, ql], 1.0, None, alu.is_ge)
                        nc.sync.dma_start(out_d.ap()[:, :, ql], sts[3][:, :, ql])
                if li == 3:
                    ql = slice(h * TH + TH // 2, (h + 1) * TH)
                    nc.vector.tensor_scalar(
                        sts[3][:, :, ql], it[:, :, ql], 1.0, None, alu.is_ge)
                    nc.sync.dma_start(out_d.ap()[:, :, ql], sts[3][:, :, ql])
                else:
                    sl = slice(h * TH, (h + 1) * TH)
                    nc.vector.tensor_scalar(
                        sts[li][:, :, sl], it[:, :, sl], 1.0, None, alu.is_ge)

            for li in range(4):
                O = O_LIST[li]
                its[li] = ip.tile([128, O, T, BS], dt.float16, tag="it",
                                  name=f"it{li}")
                sts[li] = sp.tile([128, O, T, BS], dt.float8e4, tag=f"s{li}",
                                  name=f"s{li}")
                vbs[li] = vp.tile([128, O, BS], dt.float16, tag=f"vb{li}",
                                  name=f"vb{li}")
                # pipeline: gemm(li,h1); gemm(li,h2) || rec(li,h1);
                # next layer's gemm h1 || rec(li,h2)
                gemm_half(li, 0)
                gemm_half(li, 1)
                rec_half(li, 0)
                rec_half(li, 1)

            if DEBUG_SPIKES:
                for li in range(2):
                    nc.sync.dma_start(dbg_d[li].ap(), sts[li][:])

    nc.compile()
    return nc


def _get_nc():
    if "nc" not in _CACHE:
        _CACHE["nc"] = _build_nc()
    return _CACHE["nc"]


def _host_inputs(x_tbf, Ws, bs):
    """Shared (weight/bias) arrays + per-core x shards, pre-laid-out."""
    f8 = ml_dtypes.float8_e4m3
    w_arrs = []
    b_cols = []
    for li in range(4):
        W = np.asarray(Ws[li], np.float32)
        b = np.asarray(bs[li], np.float32)
        O = O_LIST[li]
        if W.shape[0] < O * 128:           # pad layer 4: 1000 -> 1024
            pad = O * 128 - W.shape[0]
            W = np.concatenate([W, np.zeros((pad, NIN), np.float32)], 0)
            b = np.concatenate([b, np.zeros(pad, np.float32)])
        # warr[o, p, kk, i, m] = WSCALE * W[o*128+m, (2kk+i)*128+p]
        w_arrs.append(np.ascontiguousarray(
            (WSCALE * W).reshape(O, 128, KT2, 2, 128)
            .transpose(0, 4, 2, 3, 1)).astype(f8))
        b_cols.append(b.reshape(O, 128))
    b_all = np.ascontiguousarray(np.concatenate(b_cols, 0).T).astype(np.float32)

    x = np.asarray(x_tbf, np.float32)
    x_shards = []
    for c in range(NCORES):
        xc = x[:, c * BS:(c + 1) * BS, :]                    # [T, BS, NIN]
        xc = xc.transpose(2, 0, 1).reshape(NIN, COLS)        # [n, t*BS+b]
        # [p, kk, i, cols]
        xc = xc.reshape(KT2, 2, 128, COLS).transpose(2, 0, 1, 3)
        x_shards.append(np.ascontiguousarray(xc).astype(f8))
    return w_arrs, b_all, x_shards


def _decode_out(oc):
    """[128, 8, T, BS] (m, o, t, b) fp8 -> [T, BS, 1000] fp32."""
    oc = np.asarray(oc).astype(np.float32)
    oc = oc.transpose(2, 3, 1, 0).reshape(T, BS, O_LIST[3] * 128)
    return oc[:, :, :1000]


def kernel(x_tbf, W1, b1, W2, b2, W3, b3, W4, b4):
    global LAST_RESULTS
    from concourse.bass_utils import run_bass_kernel_spmd

    nc = _get_nc()
    w_arrs, b_all, x_shards = _host_inputs(
        x_tbf, [W1, W2, W3, W4], [b1, b2, b3, b4])

    in_maps = []
    for c in range(NCORES):
        m = {"x": x_shards[c], "bias": b_all}
        for li in range(4):
            m[f"w{li}"] = w_arrs[li]
        in_maps.append(m)

    res = run_bass_kernel_spmd(nc, in_maps, core_ids=list(range(NCORES)),
                               trace=TRACE)
    LAST_RESULTS = res

    out = np.empty((T, B, 1000), np.float32)
    for c in range(NCORES):
        out[:, c * BS:(c + 1) * BS, :] = _decode_out(res.results[c]["out"])
    return out


# revision 19
# speedup vs baseline: 1.7415x; 1.0152x over previous
"""Trainium2 Bass kernel for a 4-layer LIF spiking net (BPSpikingNet).

Reference semantics (per timestep t, per layer l):
    i = h @ W_l.T + b_l
    v = v - v/tau + i          (tau=2  ->  v = 0.5*v + i)
    s = (v >= 1.0)
    v = (1-s) * v              (hard reset to 0)
    h = s
Output = layer-4 spike train, shape [T=32, B=128, 1000], fp32.

Strategy:
  * Data-parallel over batch: B=128 -> 16 samples per core across 8 cores.
  * Layer-by-layer: layer l's matmul input (spikes of l-1) is fully known
    once l-1's recurrence is done, so each layer is ONE dense GEMM over all
    T*Bs = 512 (t,b) columns (weight-stationary), followed by a 32-step
    elementwise LIF recurrence.
  * fp8e4 (e4m3) matmuls in DoubleRow perf mode: each instruction contracts
    TWO 128-row k-tiles (lhsT [128,2,128], rhs [128,2,256]) at 2x PE
    throughput. Spikes are exact in fp8; weights are pre-scaled x8 on the
    host (better e4m3 coverage of the xavier range) and the PSUM eviction
    applies the inverse scale 1/8. fp32 PSUM accumulate.
  * k-major state layout [128, O, T, BS]: PSUM eviction is a flat [128,256]
    ACT copy (bias + 0.125 scale), and the next layer's DoubleRow rhs is a
    clean [128, 2, 256] slice of the fp8 spike tile.
  * fp16 recurrence: charge writes the charged potential in-place into
    it[:, :, t] (2 DVE scalar_tensor_tensor ops per step), and spikes for a
    whole half are extracted afterwards with one is_ge op.
"""

import numpy as np
import ml_dtypes

T = 32
B = 128
NCORES = 8
BS = B // NCORES          # 16 samples per core
COLS = T * BS             # 512 (t,b) columns per core
NIN = 2048
KT2 = NIN // 256          # 8 k-PAIRS (DoubleRow: 2 k-tiles per matmul)
O_LIST = [16, 16, 16, 8]  # output 128-tiles per layer (layer 4 padded 1000->1024)
BOFF = [0, 16, 32, 48]    # bias column offset per layer
NB = sum(O_LIST)          # 56 bias columns
WSCALE = 8.0              # host-side weight scale (inverse applied at eviction)

_CACHE = {}

TRACE = False             # set True (from test.py) to capture an NTFF profile
LAST_RESULTS = None       # BassKernelResults of the most recent run
DEBUG_SPIKES = False      # sim-only: dump layer-1/2 spike tiles to dram


def _build_nc():
    import concourse.mybir as mybir
    import concourse.tile as tile
    from concourse import bacc

    dt = mybir.dt
    alu = mybir.AluOpType
    DR = mybir.MatmulPerfMode.DoubleRow

    nc = bacc.Bacc("TRN2", target_bir_lowering=False, debug=False,
                   num_devices=NCORES)

    x_d = nc.dram_tensor("x", [128, KT2, 2, COLS], dt.float8e4,
                         kind="ExternalInput")
    w_d = [
        nc.dram_tensor(f"w{li}", [O_LIST[li], 128, KT2, 2, 128], dt.float8e4,
                       kind="ExternalInput")
        for li in range(4)
    ]
    b_d = nc.dram_tensor("bias", [128, NB], dt.float32, kind="ExternalInput")
    out_d = nc.dram_tensor("out", [128, O_LIST[3], T, BS], dt.float8e4,
                           kind="ExternalOutput")
    dbg_d = None
    if DEBUG_SPIKES:
        dbg_d = [nc.dram_tensor(f"dbg{li}", [128, O_LIST[li], T, BS],
                                dt.float8e4, kind="ExternalOutput")
                 for li in range(2)]

    TH = T // 2           # 16 timesteps per half
    HC = TH * BS          # 256 columns per half

    with tile.TileContext(nc) as tc:
        with (
            tc.tile_pool(name="xp", bufs=1) as xp,
            tc.tile_pool(name="sp", bufs=1) as sp,
            tc.tile_pool(name="ip", bufs=2) as ip,
            tc.tile_pool(name="wp", bufs=12) as wp,
            tc.tile_pool(name="vp", bufs=1) as vp,
            tc.tile_pool(name="bp", bufs=1) as bp,
            tc.tile_pool(name="ps", bufs=4, space="PSUM") as ps,
        ):
            # x in 8 chunks (one per k-pair) on the gpsimd DMA queue
            # (weights go on sync's), so the first matmul's two dependencies
            # transfer in parallel
            xq = []
            for c in range(KT2):
                xc = xp.tile([128, 2, COLS], dt.float8e4, tag=f"x{c}")
                nc.gpsimd.dma_start(xc[:], x_d.ap()[:, c])
                xq.append(xc)
            bt = bp.tile([128, NB], dt.float32)
            nc.gpsimd.dma_start(bt[:], b_d.ap())



            its = [None] * 4
            sts = [None] * 4
            vbs = [None] * 4

            def gemm_half(li, h):
                O = O_LIST[li]
                it = its[li]
                for o in range(O):
                    wt = wp.tile([128, KT2, 2, 128], dt.float8e4, tag="wt")
                    if li == 0 and h == 0 and o == 0:
                        # split the very first weight DMA so matmul 0 starts
                        # after half the tile has landed
                        nc.sync.dma_start(wt[:, :KT2 // 2],
                                          w_d[0].ap()[0, :, :KT2 // 2])
                        nc.sync.dma_start(wt[:, KT2 // 2:],
                                          w_d[0].ap()[0, :, KT2 // 2:])
                    else:
                        nc.sync.dma_start(wt[:], w_d[li].ap()[o])
                    acc = ps.tile([128, HC], dt.float32, tag="acc")
                    for kk in range(KT2):
                        if li == 0:
                            rhs = xq[kk][:, :, h * HC:(h + 1) * HC]
                        else:
                            rhs = sts[li - 1][:, 2 * kk:2 * kk + 2,
                                              h * TH:(h + 1) * TH, :]
                        nc.tensor.matmul(acc[:], wt[:, kk], rhs,
                                         start=(kk == 0), stop=(kk == KT2 - 1),
                                         perf_mode=DR)
                    # PSUM -> SBUF eviction: flat [128, 256] copy with bias add
                    # and the 1/WSCALE de-scale, cast to fp16
                    bias_ap = bt[:, BOFF[li] + o:BOFF[li] + o + 1]
                    nc.scalar.activation(
                        it[:, o, h * TH:(h + 1) * TH, :], acc[:],
                        mybir.ActivationFunctionType.Identity,
                        bias=bias_ap, scale=1.0 / WSCALE)

            def rec_half(li, h):
                # charge in place (it[:, :, t] becomes the charged potential);
                # only the reset state vb carries between steps
                it, vb = its[li], vbs[li]
                # Guard: a 1-column no-op (it_c = 0*s_c + it_c) that makes this
                # chain's first touch of it[:, :, first_t] READ the previous
                # extraction's output. The tile scheduler otherwise interleaves
                # this chain's ops (which wait on this gemm's evictions) into
                # the previous chain on the in-order DVE queue, head-of-line
                # blocking the previous extraction and stalling the PE.
                if li > 0 or h > 0:
                    if h == 1:
                        # previous DVE milestone: this layer's h0 extraction
                        g_src = sts[li][:, 0, TH // 2 if li == 3 else 0, 0:1]
                    else:
                        # previous layer's h1 extraction
                        g_src = sts[li - 1][:, 0, TH, 0:1]
                    g_dst = it[:, 0, h * TH, 0:1]
                    nc.vector.scalar_tensor_tensor(
                        g_dst, g_src, 0.0, g_dst, alu.mult, alu.add)
                for t in range(h * TH, (h + 1) * TH):
                    if t > 0:
                        nc.vector.scalar_tensor_tensor(
                            it[:, :, t], vb[:], 0.5, it[:, :, t],
                            alu.mult, alu.add)
                    # t == 0: v=0, so the charged potential is i_0 (already in
                    # place); the reset below also initializes vb (no memset).
                    nc.vector.scalar_tensor_tensor(
                        vb[:], it[:, :, t], 1.0, it[:, :, t], alu.is_lt, alu.mult)
                    if li == 3 and t == h * TH + TH // 2 - 1:
                        # output layer: extract+ship the finished quarter while
                        # the chain continues, so the tail only waits on 8 steps
                        ql = slice(h * TH, t + 1)
                        nc.vector.tensor_scalar(
                            sts[3][:, :, ql], it[:, :, ql], 1.0, None, alu.is_ge)
                        nc.sync.dma_start(out_d.ap()[:, :, ql], sts[3][:, :, ql])
                if li == 3:
                    ql = slice(h * TH + TH // 2, (h + 1) * TH)
                    nc.vector.tensor_scalar(
                        sts[3][:, :, ql], it[:, :, ql], 1.0, None, alu.is_ge)
                    nc.sync.dma_start(out_d.ap()[:, :, ql], sts[3][:, :, ql])
                else:
                    sl = slice(h * TH, (h + 1) * TH)
                    nc.vector.tensor_scalar(
                        sts[li][:, :, sl], it[:, :, sl], 1.0, None, alu.is_ge)

            for li in range(4):
                O = O_LIST[li]
                its[li] = ip.tile([128, O, T, BS], dt.float16, tag="it",
                                  name=f"it{li}")
                sts[li] = sp.tile([128, O, T, BS], dt.float8e4, tag=f"s{li}",
                                  name=f"s{li}")
                vbs[li] = vp.tile([128, O, BS], dt.float16, tag=f"vb{li}",
                                  name=f"vb{li}")
                # pipeline: gemm(li,h1); gemm(li,h2) || rec(li,h1);
                # next layer's gemm h1 || rec(li,h2)
                gemm_half(li, 0)
                gemm_half(li, 1)
                rec_half(li, 0)
                rec_half(li, 1)

            if DEBUG_SPIKES:
                for li in range(2):
                    nc.sync.dma_start(dbg_d[li].ap(), sts[li][:])

    nc.compile()
    return nc


def _get_nc():
    if "nc" not in _CACHE:
        _CACHE["nc"] = _build_nc()
    return _CACHE["nc"]


def _host_inputs(x_tbf, Ws, bs):
    """Shared (weight/bias) arrays + per-core x shards, pre-laid-out."""
    f8 = ml_dtypes.float8_e4m3
    w_arrs = []
    b_cols = []
    for li in range(4):
        W = np.asarray(Ws[li], np.float32)
        b = np.asarray(bs[li], np.float32)
        O = O_LIST[li]
        if W.shape[0] < O * 128:           # pad layer 4: 1000 -> 1024
            pad = O * 128 - W.shape[0]
            W = np.concatenate([W, np.zeros((pad, NIN), np.float32)], 0)
            b = np.concatenate([b, np.zeros(pad, np.float32)])
        # warr[o, p, kk, i, m] = WSCALE * W[o*128+m, (2kk+i)*128+p]
        w_arrs.append(np.ascontiguousarray(
            (WSCALE * W).reshape(O, 128, KT2, 2, 128)
            .transpose(0, 4, 2, 3, 1)).astype(f8))
        b_cols.append(b.reshape(O, 128))
    b_all = np.ascontiguousarray(np.concatenate(b_cols, 0).T).astype(np.float32)

    x = np.asarray(x_tbf, np.float32)
    x_shards = []
    for c in range(NCORES):
        xc = x[:, c * BS:(c + 1) * BS, :]                    # [T, BS, NIN]
        xc = xc.transpose(2, 0, 1).reshape(NIN, COLS)        # [n, t*BS+b]
        # [p, kk, i, cols]
        xc = xc.reshape(KT2, 2, 128, COLS).transpose(2, 0, 1, 3)
        x_shards.append(np.ascontiguousarray(xc).astype(f8))
    return w_arrs, b_all, x_shards


def _decode_out(oc):
    """[128, 8, T, BS] (m, o, t, b) fp8 -> [T, BS, 1000] fp32."""
    oc = np.asarray(oc).astype(np.float32)
    oc = oc.transpose(2, 3, 1, 0).reshape(T, BS, O_LIST[3] * 128)
    return oc[:, :, :1000]


def kernel(x_tbf, W1, b1, W2, b2, W3, b3, W4, b4):
    global LAST_RESULTS
    from concourse.bass_utils import run_bass_kernel_spmd

    nc = _get_nc()
    w_arrs, b_all, x_shards = _host_inputs(
        x_tbf, [W1, W2, W3, W4], [b1, b2, b3, b4])

    in_maps = []
    for c in range(NCORES):
        m = {"x": x_shards[c], "bias": b_all}
        for li in range(4):
            m[f"w{li}"] = w_arrs[li]
        in_maps.append(m)

    res = run_bass_kernel_spmd(nc, in_maps, core_ids=list(range(NCORES)),
                               trace=TRACE)
    LAST_RESULTS = res

    out = np.empty((T, B, 1000), np.float32)
    for c in range(NCORES):
        out[:, c * BS:(c + 1) * BS, :] = _decode_out(res.results[c]["out"])
    return out
